# revision 1
# baseline (speedup 1.0000x reference)
"""Trainium2 Bass kernel for nn_CombinedLoss (pose + point-cloud + flow loss).

Self-contained: accepts FULL inputs, shards across 8 NeuronCores internally,
returns the FULL output (5-tuple of f32 scalars, matching the reference).

Sharding strategy:
  - flow tensors  [B,1000,2,32,64]: sharded along the 1000-iteration axis
    (125 iters/core), viewed as rows=(b,t) x free=(c*h*w).
  - point_clouds  [B,4,N]: sharded along N (12500 pts/core), batch-stacked
    into [16, 12500] so one matmul handles all 4 batches.
  - tiny pose tensors: replicated; every core computes the same pose scalars.
Each core emits 5 partial scalars; the host sums partials across cores
(the all-reduce) and takes core 0's value for the replicated pose terms.
"""

import os

import numpy as np

import concourse.bass as bass
import concourse.bacc as bacc
import concourse.mybir as mybir
import concourse.tile as tile

N_CORES = 8
B = 4
N_PTS = 100000
N_ITERS = 1000
H, W = 32, 64
GAMMA = 0.8

T_PER_CORE = N_ITERS // N_CORES          # 125
ROWS = B * T_PER_CORE                    # 500 flow rows per core, b-major
FREE2 = 2 * H * W                        # 4096 (pred/gt row length)
FREE1 = H * W                            # 2048 (valid row length)
FLOW_MEAN_DEN = B * 2 * H * W            # 16384 (mean denominator per iter)
PTS_PER_CORE = N_PTS // N_CORES          # 12500
PC_GROUPS = 8                            # point groups -> 128 matmul rows
PC_COLS = 1568                           # padded 12544 / 8 groups
PAD_N = PC_GROUPS * PC_COLS              # 12544 (pads with zero points)
PC_CHUNK = 500                           # 25 matmul chunks per core
N_CHUNKS = PTS_PER_CORE // PC_CHUNK

F32 = mybir.dt.float32
BF16 = mybir.dt.bfloat16
AF = mybir.ActivationFunctionType
OP = mybir.AluOpType
AX = mybir.AxisListType

HALF_PI = float(np.pi / 2.0)


def build_nc():
    nc = bacc.Bacc("TRN2", target_bir_lowering=False, debug=False,
                   num_devices=N_CORES)

    pg = nc.dram_tensor("pg", [ROWS, 2 * FREE2], BF16, kind="ExternalInput")
    valid = nc.dram_tensor("valid", [ROWS, FREE1], BF16, kind="ExternalInput")
    wrow = nc.dram_tensor("wrow", [ROWS, 1], F32, kind="ExternalInput")
    pc = nc.dram_tensor("pc", [16 * PC_GROUPS, PC_COLS], F32, kind="ExternalInput")
    smalls = nc.dram_tensor("smalls", [B, 14], F32, kind="ExternalInput")
    out = nc.dram_tensor("out", [1, 5], F32, kind="ExternalOutput")

    with tile.TileContext(nc) as tc:
        _body(nc, tc, pg, valid, wrow, pc, smalls, out)
    nc.compile()
    return nc


def _body(nc, tc, pg, valid, wrow, pc, smalls, out):
    with (
        tc.tile_pool(name="small", bufs=1) as small,
        tc.tile_pool(name="flow", bufs=4) as flow,
        tc.tile_pool(name="pcpool", bufs=1) as pcpool,
        tc.tile_pool(name="pwork", bufs=3) as pwork,
        tc.tile_pool(name="psum_d", bufs=2, space="PSUM") as psum_d,
        tc.tile_pool(name="psum_e", bufs=2, space="PSUM") as psum_e,
        tc.tile_pool(name="psum_s", bufs=1, space="PSUM") as psum_s,
        tc.tile_pool(name="dram", bufs=1, space="DRAM") as dram,
    ):
        cnt = [0]

        def st(p_, f_, tag=None, dt=F32):
            cnt[0] += 1
            nm = tag or f"s{cnt[0]}"
            return small.tile([p_, f_], dt, name=nm, tag=nm)

        # ---------------- load tiny inputs (packed, one DMA) --------------
        sm = st(B, 14, tag="sm")
        nc.sync.dma_start(sm[:], smalls[:])
        tt_s, tr_s, te_s, re_s = sm[:, 0:3], sm[:, 3:7], sm[:, 7:10], sm[:, 10:14]

        # ---------------- loss_transl (smooth L1) ----------------
        d = st(B, 3)
        nc.gpsimd.tensor_sub(d[:], te_s, tt_s)
        a = st(B, 3)
        nc.scalar.activation(a[:], d[:], AF.Abs)
        d2 = st(B, 3)
        nc.gpsimd.tensor_mul(d2[:], d[:], d[:])
        half_d2 = st(B, 3)
        nc.gpsimd.tensor_scalar(half_d2[:], d2[:], 0.5, None, OP.mult)
        am = st(B, 3)
        nc.gpsimd.tensor_scalar(am[:], a[:], 0.5, None, OP.subtract)
        mlt = st(B, 3, dt=mybir.dt.int32)
        nc.vector.tensor_scalar(mlt[:], a[:], 1.0, None, OP.is_lt)
        sl1 = st(B, 3)
        nc.vector.select(sl1[:], mlt[:], half_d2[:], am[:])
        lt_row = st(B, 1)  # per-batch smooth-l1 row sums
        nc.vector.tensor_reduce(lt_row[:], sl1[:], axis=AX.X, op=OP.add)

        # ---------------- loss_rot (quaternion distance, RAW quats) --------
        # t = q * conj(r), q = rot_err, r = target_rot
        P0 = st(B, 4)
        P1 = st(B, 4)
        P2 = st(B, 4)
        P3 = st(B, 4)
        nc.gpsimd.tensor_scalar(P0[:], tr_s, sm[:, 10:11], None, OP.mult)
        nc.gpsimd.tensor_scalar(P1[:], tr_s, sm[:, 11:12], None, OP.mult)
        nc.gpsimd.tensor_scalar(P2[:], tr_s, sm[:, 12:13], None, OP.mult)
        nc.gpsimd.tensor_scalar(P3[:], tr_s, sm[:, 13:14], None, OP.mult)
        tw = st(B, 1)
        tx = st(B, 1)
        ty = st(B, 1)
        tz = st(B, 1)
        # tw =  P0.w + P1.x + P2.y + P3.z
        nc.gpsimd.tensor_add(tw[:], P0[:, 0:1], P1[:, 1:2])
        nc.gpsimd.tensor_add(tw[:], tw[:], P2[:, 2:3])
        nc.gpsimd.tensor_add(tw[:], tw[:], P3[:, 3:4])
        # tx = -P0.x + P1.w + P3.y - P2.z
        nc.gpsimd.tensor_sub(tx[:], P1[:, 0:1], P0[:, 1:2])
        nc.gpsimd.tensor_add(tx[:], tx[:], P3[:, 2:3])
        nc.gpsimd.tensor_sub(tx[:], tx[:], P2[:, 3:4])
        # ty = -P0.y + P1.z + P2.w - P3.x
        nc.gpsimd.tensor_sub(ty[:], P2[:, 0:1], P0[:, 2:3])
        nc.gpsimd.tensor_add(ty[:], ty[:], P1[:, 3:4])
        nc.gpsimd.tensor_sub(ty[:], ty[:], P3[:, 1:2])
        # tz = -P0.z - P1.y + P2.x + P3.w
        nc.gpsimd.tensor_sub(tz[:], P2[:, 1:2], P0[:, 3:4])
        nc.gpsimd.tensor_add(tz[:], tz[:], P3[:, 0:1])
        nc.gpsimd.tensor_sub(tz[:], tz[:], P1[:, 2:3])
        vn2 = st(B, 1)
        nc.gpsimd.tensor_mul(vn2[:], tx[:], tx[:])
        nc.vector.scalar_tensor_tensor(vn2[:], ty[:], ty[:], vn2[:], OP.mult, OP.add)
        nc.vector.scalar_tensor_tensor(vn2[:], tz[:], tz[:], vn2[:], OP.mult, OP.add)
        vn = st(B, 1)
        nc.scalar.activation(vn[:], vn2[:], AF.Sqrt)
        aw = st(B, 1)
        nc.scalar.activation(aw[:], tw[:], AF.Abs)
        # atan2(vn, aw), both >= 0: use atan of the <=1 ratio
        mx = st(B, 1)
        nc.vector.tensor_max(mx[:], vn[:], aw[:])
        mn = st(B, 1)
        nc.vector.tensor_tensor(mn[:], vn[:], aw[:], OP.min)
        rec = st(B, 1)
        nc.vector.reciprocal(rec[:], mx[:])
        ratio = st(B, 1)
        nc.gpsimd.tensor_mul(ratio[:], mn[:], rec[:])
        ang = st(B, 1)
        nc.scalar.activation(ang[:], ratio[:], AF.Arctan)
        mflip = st(B, 1, dt=mybir.dt.int32)  # vn > aw -> angle is pi/2 - atan(aw/vn)
        nc.vector.tensor_tensor(mflip[:], vn[:], aw[:], OP.is_gt)
        alt = st(B, 1)
        nc.gpsimd.tensor_scalar(alt[:], ang[:], -1.0, HALF_PI, OP.mult, OP.add)
        rot = st(B, 1)  # atan2 per batch
        nc.vector.select(rot[:], mflip[:], alt[:], ang[:])

        # ---------------- normalized quaternions ----------------
        def qnormalize(q_s):
            sq = st(B, 4)
            nc.gpsimd.tensor_mul(sq[:], q_s[:], q_s[:])
            n2 = st(B, 1)
            nc.vector.tensor_reduce(n2[:], sq[:], axis=AX.X, op=OP.add)
            nr = st(B, 1)
            nc.scalar.activation(nr[:], n2[:], AF.Sqrt)
            inv = st(B, 1)
            nc.vector.reciprocal(inv[:], nr[:])
            qn = st(B, 4)
            nc.gpsimd.tensor_scalar(qn[:], q_s[:], inv[:], None, OP.mult)
            return qn

        e = qnormalize(re_s)   # normalized rot_err
        f = qnormalize(tr_s)   # normalized target_rot

        # qm = conj(e) x f  (so R(qm) = R(e)^T R(f))
        F0 = st(B, 4)
        F1 = st(B, 4)
        F2 = st(B, 4)
        F3 = st(B, 4)
        nc.gpsimd.tensor_scalar(F0[:], f[:], e[:, 0:1], None, OP.mult)
        nc.gpsimd.tensor_scalar(F1[:], f[:], e[:, 1:2], None, OP.mult)
        nc.gpsimd.tensor_scalar(F2[:], f[:], e[:, 2:3], None, OP.mult)
        nc.gpsimd.tensor_scalar(F3[:], f[:], e[:, 3:4], None, OP.mult)
        Q = st(B, 4)  # qm = (gw, gx, gy, gz)
        # gw = F0.w + F1.x + F2.y + F3.z
        nc.gpsimd.tensor_add(Q[:, 0:1], F0[:, 0:1], F1[:, 1:2])
        nc.gpsimd.tensor_add(Q[:, 0:1], Q[:, 0:1], F2[:, 2:3])
        nc.gpsimd.tensor_add(Q[:, 0:1], Q[:, 0:1], F3[:, 3:4])
        # gx = F0.x - F1.w - F2.z + F3.y
        nc.gpsimd.tensor_sub(Q[:, 1:2], F0[:, 1:2], F1[:, 0:1])
        nc.gpsimd.tensor_sub(Q[:, 1:2], Q[:, 1:2], F2[:, 3:4])
        nc.gpsimd.tensor_add(Q[:, 1:2], Q[:, 1:2], F3[:, 2:3])
        # gy = F0.y + F1.z - F2.w - F3.x
        nc.gpsimd.tensor_add(Q[:, 2:3], F0[:, 2:3], F1[:, 3:4])
        nc.gpsimd.tensor_sub(Q[:, 2:3], Q[:, 2:3], F2[:, 0:1])
        nc.gpsimd.tensor_sub(Q[:, 2:3], Q[:, 2:3], F3[:, 1:2])
        # gz = F0.z - F1.y + F2.x - F3.w
        nc.gpsimd.tensor_sub(Q[:, 3:4], F0[:, 3:4], F1[:, 2:3])
        nc.gpsimd.tensor_add(Q[:, 3:4], Q[:, 3:4], F2[:, 1:2])
        nc.gpsimd.tensor_sub(Q[:, 3:4], Q[:, 3:4], F3[:, 0:1])

        # ---------------- A = M3 - I entries, E layout [B, 4j+i] ----------
        G1 = st(B, 4)
        G2 = st(B, 4)
        G3 = st(B, 4)
        nc.gpsimd.tensor_scalar(G1[:], Q[:], Q[:, 1:2], None, OP.mult)
        nc.gpsimd.tensor_scalar(G2[:], Q[:], Q[:, 2:3], None, OP.mult)
        nc.gpsimd.tensor_scalar(G3[:], Q[:], Q[:, 3:4], None, OP.mult)
        E = st(B, 16)
        nc.gpsimd.memset(E[:], 0.0)

        def emit(col, p_a, p_b, sub, scale2, plus1=False):
            s = st(B, 1)
            if sub:
                nc.gpsimd.tensor_sub(s[:], p_a, p_b)
            else:
                nc.gpsimd.tensor_add(s[:], p_a, p_b)
            if plus1:
                nc.gpsimd.tensor_scalar(E[:, col:col + 1], s[:], scale2, 1.0,
                                     OP.mult, OP.add)
            else:
                nc.gpsimd.tensor_scalar(E[:, col:col + 1], s[:], scale2, None,
                                     OP.mult)

        # wx=G1[:,0] x2=G1[:,1] xy=G1[:,2] xz=G1[:,3]
        # wy=G2[:,0]            y2=G2[:,2] yz=G2[:,3]
        # wz=G3[:,0]            z2=G3[:,3]
        emit(0, G2[:, 2:3], G3[:, 3:4], False, -2.0)            # A00=-2(y2+z2)
        emit(5, G1[:, 1:2], G3[:, 3:4], False, -2.0)            # A11=-2(x2+z2)
        emit(10, G1[:, 1:2], G2[:, 2:3], False, -2.0)           # A22=-2(x2+y2)
        emit(4, G1[:, 2:3], G3[:, 0:1], True, 2.0)              # A01=2(xy-wz)
        emit(8, G1[:, 3:4], G2[:, 0:1], False, 2.0)             # A02=2(xz+wy)
        emit(1, G1[:, 2:3], G3[:, 0:1], False, 2.0)             # A10=2(xy+wz)
        emit(9, G2[:, 3:4], G1[:, 0:1], True, 2.0)              # A12=2(yz-wx)
        emit(2, G1[:, 3:4], G2[:, 0:1], True, 2.0)              # A20=2(xz-wy)
        emit(6, G2[:, 3:4], G1[:, 0:1], False, 2.0)             # A21=2(yz+wx)

        # translation column: Mt = R(e)^T (tt - te) into E[:, 12:15]
        Hx = st(B, 4)
        Hy = st(B, 4)
        Hz = st(B, 4)
        nc.gpsimd.tensor_scalar(Hx[:], e[:], e[:, 1:2], None, OP.mult)
        nc.gpsimd.tensor_scalar(Hy[:], e[:], e[:, 2:3], None, OP.mult)
        nc.gpsimd.tensor_scalar(Hz[:], e[:], e[:, 3:4], None, OP.mult)
        row0 = st(B, 3)
        row1 = st(B, 3)
        row2 = st(B, 3)

        def rentry(dst, p_a, p_b, sub, scale2, plus1):
            s = st(B, 1)
            if sub:
                nc.gpsimd.tensor_sub(s[:], p_a, p_b)
            else:
                nc.gpsimd.tensor_add(s[:], p_a, p_b)
            if plus1:
                nc.gpsimd.tensor_scalar(dst, s[:], scale2, 1.0, OP.mult, OP.add)
            else:
                nc.gpsimd.tensor_scalar(dst, s[:], scale2, None, OP.mult)

        # R(e) rows: wx=Hx[:,0] x2=Hx[:,1] xy=Hx[:,2] xz=Hx[:,3]
        #            wy=Hy[:,0] y2=Hy[:,2] yz=Hy[:,3]  wz=Hz[:,0] z2=Hz[:,3]
        rentry(row0[:, 0:1], Hy[:, 2:3], Hz[:, 3:4], False, -2.0, True)  # 1-2(y2+z2)
        rentry(row0[:, 1:2], Hx[:, 2:3], Hz[:, 0:1], True, 2.0, False)   # 2(xy-wz)
        rentry(row0[:, 2:3], Hx[:, 3:4], Hy[:, 0:1], False, 2.0, False)  # 2(xz+wy)
        rentry(row1[:, 0:1], Hx[:, 2:3], Hz[:, 0:1], False, 2.0, False)  # 2(xy+wz)
        rentry(row1[:, 1:2], Hx[:, 1:2], Hz[:, 3:4], False, -2.0, True)  # 1-2(x2+z2)
        rentry(row1[:, 2:3], Hy[:, 3:4], Hx[:, 0:1], True, 2.0, False)   # 2(yz-wx)
        rentry(row2[:, 0:1], Hx[:, 3:4], Hy[:, 0:1], True, 2.0, False)   # 2(xz-wy)
        rentry(row2[:, 1:2], Hy[:, 3:4], Hx[:, 0:1], False, 2.0, False)  # 2(yz+wx)
        rentry(row2[:, 2:3], Hx[:, 1:2], Hy[:, 2:3], False, -2.0, True)  # 1-2(x2+y2)

        u = st(B, 3)
        nc.gpsimd.tensor_sub(u[:], tt_s, te_s)
        nc.gpsimd.tensor_scalar(E[:, 12:15], row0[:], u[:, 0:1], None, OP.mult)
        nc.vector.scalar_tensor_tensor(E[:, 12:15], row1[:], u[:, 1:2],
                                       E[:, 12:15], OP.mult, OP.add)
        nc.vector.scalar_tensor_tensor(E[:, 12:15], row2[:], u[:, 2:3],
                                       E[:, 12:15], OP.mult, OP.add)

        # --------- build lhsT2 [128,128]: A_b[i,j] at (16g+4b+j, 16g+4b+i) --
        # one zero-fill DMA + 8 strided scatter DMAs (one per point-group g)
        # through a DRAM bounce, then a single load.  The diagonal layout is
        # not expressible with rearrange, so the destination AP is built
        # directly: addr = 2064*g + 516*b + 128*j + i.
        z128 = st(128, 128, tag="z128")
        nc.gpsimd.memset(z128[:], 0.0)
        l2d = dram.tile([128, 128], F32)
        nc.gpsimd.dma_start(l2d[:], z128[:])
        e_view = E[:].rearrange("b (j i) -> b j i", i=4)
        l2d_ap = l2d[:]
        for g in range(PC_GROUPS):
            dst = bass.AP(l2d_ap.tensor, 2064 * g,
                          [[516, 4], [128, 4], [1, 4]])
            nc.gpsimd.dma_start(dst, e_view)
        lhsT2 = st(128, 128, tag="lhsT2")
        nc.gpsimd.dma_start(lhsT2[:], l2d[:])

        # lhsT3 [128,32] static: ones at (16g+4b+i, 4g+b) -- coordinate sum
        import ml_dtypes
        l3_np = np.zeros((128, 32), dtype=ml_dtypes.bfloat16)
        for g in range(PC_GROUPS):
            for b in range(B):
                for i in range(4):
                    l3_np[16 * g + 4 * b + i, 4 * g + b] = 1.0
        l3_dram = nc.inline_tensor(np.asarray(l3_np), name="l3_const")
        lhsT3 = st(128, 32, tag="lhsT3", dt=BF16)
        nc.gpsimd.dma_start(lhsT3[:], l3_dram[:])

        # ---------------- point-cloud: K=128 matmuls over [128,1568] ------
        pcp = pcpool.tile([128, PC_COLS], F32, tag="pcp")
        nc.sync.dma_start(pcp[:], pc[:])
        acc32 = st(32, 1, tag="acc32")
        nc.gpsimd.memset(acc32[:], 0.0)
        dsq = pcpool.tile([128, PC_COLS], BF16, tag="dsq")
        col_chunks = [(0, 512), (512, 1024), (1024, 1536), (1536, PC_COLS)]
        for c0, c1 in col_chunks:
            dps = psum_d.tile([128, 512], F32, tag="dps")
            nc.tensor.matmul(dps[:, :c1 - c0], lhsT2[:], pcp[:, c0:c1],
                             start=True, stop=True)
            nc.scalar.activation(dsq[:, c0:c1], dps[:, :c1 - c0], AF.Square)
        for c0, c1 in col_chunks:
            e2 = psum_e.tile([32, 512], F32, tag="e2")
            nc.tensor.matmul(e2[:, :c1 - c0], lhsT3[:], dsq[:, c0:c1],
                             start=True, stop=True)
            errt = pwork.tile([32, 512], F32, tag="errt")
            ers = pwork.tile([32, 1], F32, tag="ers")
            nc.scalar.activation(errt[:, :c1 - c0], e2[:, :c1 - c0], AF.Sqrt,
                                 accum_out=ers[:])
            nc.vector.tensor_add(acc32[:], acc32[:], ers[:])

        # ---------------- flow loop ----------------
        acc128 = st(128, 1, tag="acc128")
        nc.gpsimd.memset(acc128[:], 0.0)
        FROWS = 64
        row_starts = list(range(0, ROWS, FROWS))
        for r0 in row_starts:
            rp = min(FROWS, ROWS - r0)
            pg_t = flow.tile([FROWS, 2 * FREE2], BF16, tag="pg")
            v_t = flow.tile([FROWS, FREE1], BF16, tag="v")
            w_t = flow.tile([FROWS, 1], F32, tag="w")
            nc.sync.dma_start(pg_t[:rp, :], pg[r0:r0 + rp, :])
            nc.sync.dma_start(v_t[:rp, :], valid[r0:r0 + rp, :])
            nc.sync.dma_start(w_t[:rp, :], wrow[r0:r0 + rp, :])
            d_t = flow.tile([FROWS, FREE2], BF16, tag="d")
            nc.vector.tensor_sub(d_t[:rp, :], pg_t[:rp, 0:FREE2],
                                 pg_t[:rp, FREE2:2 * FREE2])
            nc.vector.tensor_mul(d_t[:rp, 0:FREE1], d_t[:rp, 0:FREE1],
                                 v_t[:rp, :])
            nc.vector.tensor_mul(d_t[:rp, FREE1:FREE2], d_t[:rp, FREE1:FREE2],
                                 v_t[:rp, :])
            rs = flow.tile([FROWS, 1], F32, tag="rs")
            nc.scalar.activation(d_t[:rp, :], d_t[:rp, :], AF.Abs,
                                 scale=w_t[:rp, :], accum_out=rs[:rp, :])
            nc.vector.tensor_add(acc128[:rp, :], acc128[:rp, :], rs[:rp, :])

        # ---------------- final reductions ----------------
        ones128 = st(128, 1, tag="ones128")
        nc.gpsimd.memset(ones128[:], 1.0)
        ones4 = st(B, 1, tag="ones4")
        nc.gpsimd.memset(ones4[:], 1.0)
        ones32 = st(32, 1, tag="ones32")
        nc.gpsimd.memset(ones32[:], 1.0)
        ps = psum_s.tile([1, 4], F32, tag="ps")
        nc.tensor.matmul(ps[:, 0:1], acc128[:], ones128[:], start=True, stop=True)
        nc.tensor.matmul(ps[:, 1:2], acc32[:], ones32[:], start=True, stop=True)
        nc.tensor.matmul(ps[:, 2:3], lt_row[:], ones4[:], start=True, stop=True)
        nc.tensor.matmul(ps[:, 3:4], rot[:], ones4[:], start=True, stop=True)

        out5 = st(1, 5, tag="out5")
        # loss_transl = sum/4 ; loss_rot = 2*sum/4 ; pc = sum/(B*N) ; flow = sum
        nc.scalar.mul(out5[:, 1:2], ps[:, 2:3], 0.25)
        nc.scalar.mul(out5[:, 2:3], ps[:, 3:4], 0.5)
        nc.scalar.mul(out5[:, 3:4], ps[:, 1:2], 1.0 / (B * N_PTS))
        nc.scalar.copy(out5[:, 4:5], ps[:, 0:1])
        t1 = st(1, 1)
        t2 = st(1, 1)
        nc.gpsimd.tensor_add(t1[:], out5[:, 1:2], out5[:, 2:3])
        nc.gpsimd.tensor_add(t2[:], out5[:, 3:4], out5[:, 4:5])
        nc.gpsimd.tensor_scalar(t1[:], t1[:], 0.5 / N_CORES, None, OP.mult)
        nc.vector.scalar_tensor_tensor(out5[:, 0:1], t2[:], 0.5, t1[:],
                                       OP.mult, OP.add)
        nc.sync.dma_start(out[:], out5[:])


_CACHE = {}
last_results = None


def _get_nc():
    if "nc" not in _CACHE:
        _CACHE["nc"] = build_nc()
    return _CACHE["nc"]


def make_in_maps(point_clouds, target_transl, target_rot, transl_err, rot_err,
                 calib_flow_pred, calib_flow_gt, flow_valid):
    point_clouds = np.asarray(point_clouds, np.float32)
    calib_flow_pred = np.asarray(calib_flow_pred, np.float32)
    calib_flow_gt = np.asarray(calib_flow_gt, np.float32)
    flow_valid = np.asarray(flow_valid, np.float32)
    tt = np.ascontiguousarray(np.asarray(target_transl, np.float32))
    tr = np.ascontiguousarray(np.asarray(target_rot, np.float32))
    te = np.ascontiguousarray(np.asarray(transl_err, np.float32))
    re = np.ascontiguousarray(np.asarray(rot_err, np.float32))

    w_full = (GAMMA ** (N_ITERS - 1 - np.arange(N_ITERS, dtype=np.float64)))
    w_full = (w_full / FLOW_MEAN_DEN).astype(np.float32)

    import ml_dtypes
    smalls = np.concatenate([tt, tr, te, re], axis=1).astype(np.float32)
    pred16 = calib_flow_pred.astype(ml_dtypes.bfloat16)
    gt16 = calib_flow_gt.astype(ml_dtypes.bfloat16)
    valid16 = flow_valid.astype(ml_dtypes.bfloat16)
    in_maps = []
    for c in range(N_CORES):
        t0, t1 = c * T_PER_CORE, (c + 1) * T_PER_CORE
        n0, n1 = c * PTS_PER_CORE, (c + 1) * PTS_PER_CORE
        p_s = pred16[:, t0:t1].reshape(ROWS, FREE2)
        g_s = gt16[:, t0:t1].reshape(ROWS, FREE2)
        in_maps.append({
            "pg": np.ascontiguousarray(
                np.concatenate([p_s, g_s], axis=1)),
            "valid": np.ascontiguousarray(
                valid16[:, t0:t1]).reshape(ROWS, FREE1),
            "wrow": np.ascontiguousarray(
                np.tile(w_full[t0:t1], B)).reshape(ROWS, 1),
            "pc": _pack_pc(point_clouds[:, :, n0:n1]),
            "smalls": smalls,
        })
    return in_maps


def _pack_pc(pc_shard):
    """[B,4,12500] -> [128,1568]: row 16g+4b+j = pc[b,j,1568g:1568(g+1)],
    zero-padded to 12544 points (zero points contribute zero error)."""
    pad = np.zeros((B, 4, PAD_N), np.float32)
    pad[:, :, :PTS_PER_CORE] = pc_shard
    v = pad.reshape(B, 4, PC_GROUPS, PC_COLS)
    return np.ascontiguousarray(
        v.transpose(2, 0, 1, 3).reshape(16 * PC_GROUPS, PC_COLS))


def combine_outputs(core_outs):
    """core_outs: [N_CORES, 5] array of per-core partials."""
    core_outs = np.asarray(core_outs, np.float32)
    total = np.float32(core_outs[:, 0].sum())
    lt = np.float32(core_outs[0, 1])
    lr = np.float32(core_outs[0, 2])
    pcb = np.float32(core_outs[:, 3].sum())
    fl = np.float32(core_outs[:, 4].sum())
    return (total, lt, lr, pcb, fl)


def _install_ntff_hook_shim():
    """bass_utils expects antenv.axon_hooks when trace=True under axon;
    this image's antenv lacks it. Provide it and register the ctypes hook."""
    import sys
    import types
    if "antenv.axon_hooks" in sys.modules:
        return
    mod = types.ModuleType("antenv.axon_hooks")
    state = {"hook": None}
    mod.set_axon_ntff_profile_hook = lambda h: state.__setitem__("hook", h)
    mod.get_axon_ntff_profile_hook = lambda: state["hook"]
    sys.modules["antenv.axon_hooks"] = mod
    try:
        import antenv
        antenv.axon_hooks = mod
    except ImportError:
        pass
    try:
        from trn_agent_boot.trn_boot import _ntff_profile_via_ctypes
        mod.set_axon_ntff_profile_hook(
            _ntff_profile_via_ctypes("/opt/axon/libaxon_pjrt.so"))
    except Exception:
        pass


def kernel(point_clouds, target_transl, target_rot, transl_err, rot_err,
           calib_flow_pred, calib_flow_gt, flow_valid):
    global last_results
    from concourse.bass_utils import run_bass_kernel_spmd

    nc = _get_nc()
    in_maps = make_in_maps(point_clouds, target_transl, target_rot,
                           transl_err, rot_err, calib_flow_pred,
                           calib_flow_gt, flow_valid)
    trace = bool(int(os.environ.get("KERNEL_TRACE", "0")))
    kwargs = {}
    if trace:
        _install_ntff_hook_shim()
        kwargs = {"trace": True, "trace_cores": list(range(N_CORES))}
    res = run_bass_kernel_spmd(nc, in_maps, core_ids=list(range(N_CORES)),
                               **kwargs)
    last_results = res
    core_outs = np.stack([res.results[c]["out"][0] for c in range(N_CORES)])
    return combine_outputs(core_outs)



# revision 5
# speedup vs baseline: 2.0517x; 2.0517x over previous
"""Trainium2 Bass kernel for nn_CombinedLoss (pose + point-cloud + flow loss).

Self-contained: accepts FULL inputs, shards across 8 NeuronCores internally,
returns the FULL output (5-tuple of f32 scalars, matching the reference).

Sharding strategy:
  - flow tensors [B,1000,2,32,64]: the loss weights gamma^(999-t) decay so
    fast that iterations with t < 872 contribute < 1e-12 relative to the f32
    result; only the last 128 iterations are loaded.  Those are sharded along
    the iteration axis (16 iters/core) and reshaped to a [128, 2048] tile
    (row = (b, t, c), cols = h*w) so all 128 partitions are used.
  - point_clouds [B,4,N]: sharded along N (12500 pts/core), packed bf16 into
    [128, 1568] so one matmul chain handles all 4 batches.
  - tiny pose tensors: replicated; every core computes the same pose scalars.
Each core emits 5 partial scalars; the host sums partials across cores
(the all-reduce) and takes core 0's value for the replicated pose terms.
"""

import os

import numpy as np

import concourse.bass as bass
import concourse.bacc as bacc
import concourse.mybir as mybir
import concourse.tile as tile

N_CORES = 8
B = 4
N_PTS = 100000
N_ITERS = 1000
H, W = 32, 64
GAMMA = 0.8

T_KEEP = 128                             # kept flow iterations (tail)
T0_GLOBAL = N_ITERS - T_KEEP             # 872
TPC = T_KEEP // N_CORES                  # 16 iters per core
FROWS = B * TPC * 2                      # 128 rows: (b, t, c)
FREE1 = H * W                            # 2048 cols (h*w)
FLOW_MEAN_DEN = B * 2 * H * W            # 16384 (mean denominator per iter)
PTS_PER_CORE = N_PTS // N_CORES          # 12500
PC_GROUPS = 8                            # point groups -> 128 matmul rows
PC_COLS = 1568                           # padded 12544 / 8 groups
PAD_N = PC_GROUPS * PC_COLS              # 12544 (pads with zero points)

F32 = mybir.dt.float32
BF16 = mybir.dt.bfloat16
AF = mybir.ActivationFunctionType
OP = mybir.AluOpType
AX = mybir.AxisListType

HALF_PI = float(np.pi / 2.0)


def build_nc():
    nc = bacc.Bacc("TRN2", target_bir_lowering=False, debug=False,
                   num_devices=N_CORES)

    pg = nc.dram_tensor("pg", [FROWS, 2 * FREE1], BF16, kind="ExternalInput")
    vv = nc.dram_tensor("vv", [FROWS, FREE1], BF16, kind="ExternalInput")
    wrow = nc.dram_tensor("wrow", [FROWS, 1], F32, kind="ExternalInput")
    pc = nc.dram_tensor("pc", [16 * PC_GROUPS, PC_COLS], BF16,
                        kind="ExternalInput")
    smalls = nc.dram_tensor("smalls", [B, 14], F32, kind="ExternalInput")
    out = nc.dram_tensor("out", [1, 5], F32, kind="ExternalOutput")

    with tile.TileContext(nc) as tc:
        _body(nc, tc, pg, vv, wrow, pc, smalls, out)
    nc.compile()
    return nc


def _body(nc, tc, pg, vv, wrow, pc, smalls, out):
    with (
        tc.tile_pool(name="small", bufs=1) as small,
        tc.tile_pool(name="flow", bufs=1) as flow,
        tc.tile_pool(name="pcpool", bufs=1) as pcpool,
        tc.tile_pool(name="pwork", bufs=3) as pwork,
        tc.tile_pool(name="psum_d", bufs=2, space="PSUM") as psum_d,
        tc.tile_pool(name="psum_e", bufs=2, space="PSUM") as psum_e,
        tc.tile_pool(name="psum_s", bufs=1, space="PSUM") as psum_s,
        tc.tile_pool(name="dram", bufs=1, space="DRAM") as dram,
    ):
        cnt = [0]

        def st(p_, f_, tag=None, dt=F32):
            cnt[0] += 1
            nm = tag or f"s{cnt[0]}"
            return small.tile([p_, f_], dt, name=nm, tag=nm)

        # ---------------- DMA triggers (spread across queues) --------------
        sm = st(B, 14, tag="sm")
        nc.sync.dma_start(sm[:], smalls[:])          # pose inputs first
        pgt = flow.tile([FROWS, 2 * FREE1], BF16, tag="pgt")
        nc.sync.dma_start(pgt[:], pg[:])
        pcp = pcpool.tile([128, PC_COLS], BF16, tag="pcp")
        nc.sync.dma_start(pcp[:], pc[:])
        vvt = flow.tile([FROWS, FREE1], BF16, tag="vvt")
        nc.scalar.dma_start(vvt[:], vv[:])
        wt = st(FROWS, 1, tag="wt")
        nc.scalar.dma_start(wt[:], wrow[:])

        # lhsT3 [128,32] static: ones at (16g+4b+i, 4g+b) -- coordinate sum
        import ml_dtypes
        l3_np = np.zeros((128, 32), dtype=ml_dtypes.bfloat16)
        for g in range(PC_GROUPS):
            for b in range(B):
                for i in range(4):
                    l3_np[16 * g + 4 * b + i, 4 * g + b] = 1.0
        l3_dram = nc.inline_tensor(np.asarray(l3_np), name="l3_const")
        lhsT3 = st(128, 32, tag="lhsT3", dt=BF16)
        nc.gpsimd.dma_start(lhsT3[:], l3_dram[:])

        # ---------------- flow (vector engine, queue head) ----------------
        # rows = (b, t, c); per row: sum_hw |pred-gt| * valid, then * w_row.
        d_t = flow.tile([FROWS, FREE1], BF16, tag="d")
        nc.vector.tensor_sub(d_t[:], pgt[:, 0:FREE1], pgt[:, FREE1:2 * FREE1])
        dv_t = flow.tile([FROWS, FREE1], BF16, tag="dv")
        nc.vector.tensor_mul(dv_t[:], d_t[:], vvt[:])
        dum = flow.tile([FROWS, FREE1], BF16, tag="dum")
        rs2 = st(FROWS, 1, tag="rs2")
        # |w*dv| = w*|dv| (w>0), summed over the free axis; emitted later on
        # the scalar queue (after the pose ACTIVATEs) -- see below.

        def flow_abs_accum():
            nc.scalar.activation(dum[:], dv_t[:], AF.Abs, scale=wt[:],
                                 accum_out=rs2[:])

        # ---------------- pose: smooth-L1 translation loss ----------------
        tt_s, tr_s, te_s, re_s = sm[:, 0:3], sm[:, 3:7], sm[:, 7:10], sm[:, 10:14]
        d = st(B, 3)
        nc.gpsimd.tensor_sub(d[:], te_s, tt_s)
        a = st(B, 3)
        nc.scalar.activation(a[:], d[:], AF.Abs)
        d2 = st(B, 3)
        nc.gpsimd.tensor_mul(d2[:], d[:], d[:])
        half_d2 = st(B, 3)
        nc.gpsimd.tensor_scalar(half_d2[:], d2[:], 0.5, None, OP.mult)
        am = st(B, 3)
        nc.gpsimd.tensor_scalar(am[:], a[:], 0.5, None, OP.subtract)
        mlt = st(B, 3, dt=mybir.dt.int32)
        nc.vector.tensor_scalar(mlt[:], a[:], 1.0, None, OP.is_lt)
        sl1 = st(B, 3)
        nc.vector.select(sl1[:], mlt[:], half_d2[:], am[:])
        lt_row = st(B, 1)  # per-batch smooth-l1 row sums
        nc.vector.tensor_reduce(lt_row[:], sl1[:], axis=AX.X, op=OP.add)

        # ---------------- loss_rot (quaternion distance, RAW quats) --------
        P0 = st(B, 4)
        P1 = st(B, 4)
        P2 = st(B, 4)
        P3 = st(B, 4)
        nc.gpsimd.tensor_scalar(P0[:], tr_s, sm[:, 10:11], None, OP.mult)
        nc.gpsimd.tensor_scalar(P1[:], tr_s, sm[:, 11:12], None, OP.mult)
        nc.gpsimd.tensor_scalar(P2[:], tr_s, sm[:, 12:13], None, OP.mult)
        nc.gpsimd.tensor_scalar(P3[:], tr_s, sm[:, 13:14], None, OP.mult)
        tw = st(B, 1)
        tx = st(B, 1)
        ty = st(B, 1)
        tz = st(B, 1)
        nc.gpsimd.tensor_add(tw[:], P0[:, 0:1], P1[:, 1:2])
        nc.gpsimd.tensor_add(tw[:], tw[:], P2[:, 2:3])
        nc.gpsimd.tensor_add(tw[:], tw[:], P3[:, 3:4])
        nc.gpsimd.tensor_sub(tx[:], P1[:, 0:1], P0[:, 1:2])
        nc.gpsimd.tensor_add(tx[:], tx[:], P3[:, 2:3])
        nc.gpsimd.tensor_sub(tx[:], tx[:], P2[:, 3:4])
        nc.gpsimd.tensor_sub(ty[:], P2[:, 0:1], P0[:, 2:3])
        nc.gpsimd.tensor_add(ty[:], ty[:], P1[:, 3:4])
        nc.gpsimd.tensor_sub(ty[:], ty[:], P3[:, 1:2])
        nc.gpsimd.tensor_sub(tz[:], P2[:, 1:2], P0[:, 3:4])
        nc.gpsimd.tensor_add(tz[:], tz[:], P3[:, 0:1])
        nc.gpsimd.tensor_sub(tz[:], tz[:], P1[:, 2:3])
        vn2 = st(B, 1)
        nc.gpsimd.tensor_mul(vn2[:], tx[:], tx[:])
        nc.vector.scalar_tensor_tensor(vn2[:], ty[:], ty[:], vn2[:], OP.mult, OP.add)
        nc.vector.scalar_tensor_tensor(vn2[:], tz[:], tz[:], vn2[:], OP.mult, OP.add)
        vn = st(B, 1)
        nc.scalar.activation(vn[:], vn2[:], AF.Sqrt)
        aw = st(B, 1)
        nc.scalar.activation(aw[:], tw[:], AF.Abs)
        mx = st(B, 1)
        nc.vector.tensor_max(mx[:], vn[:], aw[:])
        mn = st(B, 1)
        nc.vector.tensor_tensor(mn[:], vn[:], aw[:], OP.min)
        rec = st(B, 1)
        nc.vector.reciprocal(rec[:], mx[:])
        ratio = st(B, 1)
        nc.gpsimd.tensor_mul(ratio[:], mn[:], rec[:])
        ang = st(B, 1)
        nc.scalar.activation(ang[:], ratio[:], AF.Arctan)
        mflip = st(B, 1, dt=mybir.dt.int32)
        nc.vector.tensor_tensor(mflip[:], vn[:], aw[:], OP.is_gt)
        alt = st(B, 1)
        nc.gpsimd.tensor_scalar(alt[:], ang[:], -1.0, HALF_PI, OP.mult, OP.add)
        rot = st(B, 1)  # atan2 per batch
        nc.vector.select(rot[:], mflip[:], alt[:], ang[:])

        # ---------------- normalized quaternions ----------------
        def qnormalize(q_s):
            sq = st(B, 4)
            nc.gpsimd.tensor_mul(sq[:], q_s[:], q_s[:])
            n2 = st(B, 1)
            nc.vector.tensor_reduce(n2[:], sq[:], axis=AX.X, op=OP.add)
            nr = st(B, 1)
            nc.scalar.activation(nr[:], n2[:], AF.Sqrt)
            inv = st(B, 1)
            nc.vector.reciprocal(inv[:], nr[:])
            qn = st(B, 4)
            nc.gpsimd.tensor_scalar(qn[:], q_s[:], inv[:], None, OP.mult)
            return qn

        e = qnormalize(re_s)   # normalized rot_err
        f = qnormalize(tr_s)   # normalized target_rot

        # qm = conj(e) x f  (so R(qm) = R(e)^T R(f))
        F0 = st(B, 4)
        F1 = st(B, 4)
        F2 = st(B, 4)
        F3 = st(B, 4)
        nc.gpsimd.tensor_scalar(F0[:], f[:], e[:, 0:1], None, OP.mult)
        nc.gpsimd.tensor_scalar(F1[:], f[:], e[:, 1:2], None, OP.mult)
        nc.gpsimd.tensor_scalar(F2[:], f[:], e[:, 2:3], None, OP.mult)
        nc.gpsimd.tensor_scalar(F3[:], f[:], e[:, 3:4], None, OP.mult)
        Q = st(B, 4)  # qm = (gw, gx, gy, gz)
        nc.gpsimd.tensor_add(Q[:, 0:1], F0[:, 0:1], F1[:, 1:2])
        nc.gpsimd.tensor_add(Q[:, 0:1], Q[:, 0:1], F2[:, 2:3])
        nc.gpsimd.tensor_add(Q[:, 0:1], Q[:, 0:1], F3[:, 3:4])
        nc.gpsimd.tensor_sub(Q[:, 1:2], F0[:, 1:2], F1[:, 0:1])
        nc.gpsimd.tensor_sub(Q[:, 1:2], Q[:, 1:2], F2[:, 3:4])
        nc.gpsimd.tensor_add(Q[:, 1:2], Q[:, 1:2], F3[:, 2:3])
        nc.gpsimd.tensor_add(Q[:, 2:3], F0[:, 2:3], F1[:, 3:4])
        nc.gpsimd.tensor_sub(Q[:, 2:3], Q[:, 2:3], F2[:, 0:1])
        nc.gpsimd.tensor_sub(Q[:, 2:3], Q[:, 2:3], F3[:, 1:2])
        nc.gpsimd.tensor_sub(Q[:, 3:4], F0[:, 3:4], F1[:, 2:3])
        nc.gpsimd.tensor_add(Q[:, 3:4], Q[:, 3:4], F2[:, 1:2])
        nc.gpsimd.tensor_sub(Q[:, 3:4], Q[:, 3:4], F3[:, 0:1])

        # ---------------- A = M3 - I entries, E layout [B, 4j+i] ----------
        G1 = st(B, 4)
        G2 = st(B, 4)
        G3 = st(B, 4)
        nc.gpsimd.tensor_scalar(G1[:], Q[:], Q[:, 1:2], None, OP.mult)
        nc.gpsimd.tensor_scalar(G2[:], Q[:], Q[:, 2:3], None, OP.mult)
        nc.gpsimd.tensor_scalar(G3[:], Q[:], Q[:, 3:4], None, OP.mult)
        E = st(B, 16)
        nc.gpsimd.memset(E[:], 0.0)

        def emit(col, p_a, p_b, sub, scale2, plus1=False):
            s = st(B, 1)
            if sub:
                nc.gpsimd.tensor_sub(s[:], p_a, p_b)
            else:
                nc.gpsimd.tensor_add(s[:], p_a, p_b)
            if plus1:
                nc.gpsimd.tensor_scalar(E[:, col:col + 1], s[:], scale2, 1.0,
                                     OP.mult, OP.add)
            else:
                nc.gpsimd.tensor_scalar(E[:, col:col + 1], s[:], scale2, None,
                                     OP.mult)

        emit(0, G2[:, 2:3], G3[:, 3:4], False, -2.0)            # A00=-2(y2+z2)
        emit(5, G1[:, 1:2], G3[:, 3:4], False, -2.0)            # A11=-2(x2+z2)
        emit(10, G1[:, 1:2], G2[:, 2:3], False, -2.0)           # A22=-2(x2+y2)
        emit(4, G1[:, 2:3], G3[:, 0:1], True, 2.0)              # A01=2(xy-wz)
        emit(8, G1[:, 3:4], G2[:, 0:1], False, 2.0)             # A02=2(xz+wy)
        emit(1, G1[:, 2:3], G3[:, 0:1], False, 2.0)             # A10=2(xy+wz)
        emit(9, G2[:, 3:4], G1[:, 0:1], True, 2.0)              # A12=2(yz-wx)
        emit(2, G1[:, 3:4], G2[:, 0:1], True, 2.0)              # A20=2(xz-wy)
        emit(6, G2[:, 3:4], G1[:, 0:1], False, 2.0)             # A21=2(yz+wx)

        # translation column: Mt = R(e)^T (tt - te) into E[:, 12:15]
        Hx = st(B, 4)
        Hy = st(B, 4)
        Hz = st(B, 4)
        nc.gpsimd.tensor_scalar(Hx[:], e[:], e[:, 1:2], None, OP.mult)
        nc.gpsimd.tensor_scalar(Hy[:], e[:], e[:, 2:3], None, OP.mult)
        nc.gpsimd.tensor_scalar(Hz[:], e[:], e[:, 3:4], None, OP.mult)
        row0 = st(B, 3)
        row1 = st(B, 3)
        row2 = st(B, 3)

        def rentry(dst, p_a, p_b, sub, scale2, plus1):
            s = st(B, 1)
            if sub:
                nc.gpsimd.tensor_sub(s[:], p_a, p_b)
            else:
                nc.gpsimd.tensor_add(s[:], p_a, p_b)
            if plus1:
                nc.gpsimd.tensor_scalar(dst, s[:], scale2, 1.0, OP.mult, OP.add)
            else:
                nc.gpsimd.tensor_scalar(dst, s[:], scale2, None, OP.mult)

        rentry(row0[:, 0:1], Hy[:, 2:3], Hz[:, 3:4], False, -2.0, True)
        rentry(row0[:, 1:2], Hx[:, 2:3], Hz[:, 0:1], True, 2.0, False)
        rentry(row0[:, 2:3], Hx[:, 3:4], Hy[:, 0:1], False, 2.0, False)
        rentry(row1[:, 0:1], Hx[:, 2:3], Hz[:, 0:1], False, 2.0, False)
        rentry(row1[:, 1:2], Hx[:, 1:2], Hz[:, 3:4], False, -2.0, True)
        rentry(row1[:, 2:3], Hy[:, 3:4], Hx[:, 0:1], True, 2.0, False)
        rentry(row2[:, 0:1], Hx[:, 3:4], Hy[:, 0:1], True, 2.0, False)
        rentry(row2[:, 1:2], Hy[:, 3:4], Hx[:, 0:1], False, 2.0, False)
        rentry(row2[:, 2:3], Hx[:, 1:2], Hy[:, 2:3], False, -2.0, True)

        u = st(B, 3)
        nc.gpsimd.tensor_sub(u[:], tt_s, te_s)
        nc.gpsimd.tensor_scalar(E[:, 12:15], row0[:], u[:, 0:1], None, OP.mult)
        nc.vector.scalar_tensor_tensor(E[:, 12:15], row1[:], u[:, 1:2],
                                       E[:, 12:15], OP.mult, OP.add)
        nc.vector.scalar_tensor_tensor(E[:, 12:15], row2[:], u[:, 2:3],
                                       E[:, 12:15], OP.mult, OP.add)

        flow_abs_accum()

        # --------- build lhsT2 [128,128] bf16: A_b[i,j] at (16g+4b+j, 16g+4b+i)
        E_bf = st(B, 16, tag="E_bf", dt=BF16)
        nc.gpsimd.tensor_scalar(E_bf[:], E[:], 1.0, None, OP.mult)
        z128 = st(128, 128, tag="z128", dt=BF16)
        nc.gpsimd.memset(z128[:], 0.0)
        l2d = dram.tile([128, 128], BF16)
        nc.gpsimd.dma_start(l2d[:], z128[:])
        e_view = E_bf[:].rearrange("b (j i) -> b j i", i=4)
        l2d_ap = l2d[:]
        for g in range(PC_GROUPS):
            dst = bass.AP(l2d_ap.tensor, 2064 * g,
                          [[516, 4], [128, 4], [1, 4]])
            nc.gpsimd.dma_start(dst, e_view)
        lhsT2 = st(128, 128, tag="lhsT2", dt=BF16)
        nc.gpsimd.dma_start(lhsT2[:], l2d[:])

        # ---------------- point-cloud: K=128 matmuls over [128,1568] ------
        acc32 = st(32, 1, tag="acc32")
        nc.gpsimd.memset(acc32[:], 0.0)
        dsq = pcpool.tile([128, PC_COLS], BF16, tag="dsq")
        col_chunks = [(0, 512), (512, 1024), (1024, 1536), (1536, PC_COLS)]
        for c0, c1 in col_chunks:
            dps = psum_d.tile([128, 512], F32, tag="dps")
            nc.tensor.matmul(dps[:, :c1 - c0], lhsT2[:], pcp[:, c0:c1],
                             start=True, stop=True)
            nc.scalar.activation(dsq[:, c0:c1], dps[:, :c1 - c0], AF.Square)
        for c0, c1 in col_chunks:
            e2 = psum_e.tile([32, 512], F32, tag="e2")
            nc.tensor.matmul(e2[:, :c1 - c0], lhsT3[:], dsq[:, c0:c1],
                             start=True, stop=True)
            errt = pwork.tile([32, 512], F32, tag="errt")
            ers = pwork.tile([32, 1], F32, tag="ers")
            nc.scalar.activation(errt[:, :c1 - c0], e2[:, :c1 - c0], AF.Sqrt,
                                 accum_out=ers[:])
            nc.vector.tensor_add(acc32[:], acc32[:], ers[:])

        # ---------------- final reductions ----------------
        ones128 = st(128, 1, tag="ones128")
        nc.gpsimd.memset(ones128[:], 1.0)
        ones4 = st(B, 1, tag="ones4")
        nc.gpsimd.memset(ones4[:], 1.0)
        ones32 = st(32, 1, tag="ones32")
        nc.gpsimd.memset(ones32[:], 1.0)
        ps = psum_s.tile([1, 4], F32, tag="ps")
        nc.tensor.matmul(ps[:, 0:1], rs2[:], ones128[:], start=True, stop=True)
        nc.tensor.matmul(ps[:, 1:2], acc32[:], ones32[:], start=True, stop=True)
        nc.tensor.matmul(ps[:, 2:3], lt_row[:], ones4[:], start=True, stop=True)
        nc.tensor.matmul(ps[:, 3:4], rot[:], ones4[:], start=True, stop=True)

        out5 = st(1, 5, tag="out5")
        # loss_transl = sum/4 ; loss_rot = 2*sum/4 ; pc = sum/(B*N) ; flow = sum
        nc.scalar.mul(out5[:, 1:2], ps[:, 2:3], 0.25)
        nc.scalar.mul(out5[:, 2:3], ps[:, 3:4], 0.5)
        nc.scalar.mul(out5[:, 3:4], ps[:, 1:2], 1.0 / (B * N_PTS))
        nc.scalar.copy(out5[:, 4:5], ps[:, 0:1])
        t1 = st(1, 1)
        t2 = st(1, 1)
        nc.gpsimd.tensor_add(t1[:], out5[:, 1:2], out5[:, 2:3])
        nc.gpsimd.tensor_add(t2[:], out5[:, 3:4], out5[:, 4:5])
        nc.gpsimd.tensor_scalar(t1[:], t1[:], 0.5 / N_CORES, None, OP.mult)
        nc.vector.scalar_tensor_tensor(out5[:, 0:1], t2[:], 0.5, t1[:],
                                       OP.mult, OP.add)
        nc.sync.dma_start(out[:], out5[:])


_CACHE = {}
last_results = None


def _get_nc():
    if "nc" not in _CACHE:
        _CACHE["nc"] = build_nc()
    return _CACHE["nc"]


def make_in_maps(point_clouds, target_transl, target_rot, transl_err, rot_err,
                 calib_flow_pred, calib_flow_gt, flow_valid):
    import ml_dtypes
    point_clouds = np.asarray(point_clouds, np.float32)
    calib_flow_pred = np.asarray(calib_flow_pred, np.float32)
    calib_flow_gt = np.asarray(calib_flow_gt, np.float32)
    flow_valid = np.asarray(flow_valid, np.float32)
    tt = np.ascontiguousarray(np.asarray(target_transl, np.float32))
    tr = np.ascontiguousarray(np.asarray(target_rot, np.float32))
    te = np.ascontiguousarray(np.asarray(transl_err, np.float32))
    re = np.ascontiguousarray(np.asarray(rot_err, np.float32))

    w_full = (GAMMA ** (N_ITERS - 1 - np.arange(N_ITERS, dtype=np.float64)))
    w_full = (w_full / FLOW_MEAN_DEN).astype(np.float32)

    smalls = np.concatenate([tt, tr, te, re], axis=1).astype(np.float32)
    in_maps = []
    for c in range(N_CORES):
        t0 = T0_GLOBAL + c * TPC
        t1 = t0 + TPC
        n0, n1 = c * PTS_PER_CORE, (c + 1) * PTS_PER_CORE
        p_s = calib_flow_pred[:, t0:t1].reshape(FROWS, FREE1)
        g_s = calib_flow_gt[:, t0:t1].reshape(FROWS, FREE1)
        vv_s = np.repeat(flow_valid[:, t0:t1], 2, axis=2).reshape(FROWS, FREE1)
        # row p = b*32 + ti*2 + k  ->  w = w_full[t0+ti]
        w_s = np.tile(np.repeat(w_full[t0:t1], 2), B).reshape(FROWS, 1)
        in_maps.append({
            "pg": np.ascontiguousarray(
                np.concatenate([p_s, g_s], axis=1)).astype(ml_dtypes.bfloat16),
            "vv": np.ascontiguousarray(vv_s).astype(ml_dtypes.bfloat16),
            "wrow": np.ascontiguousarray(w_s.astype(np.float32)),
            "pc": _pack_pc(point_clouds[:, :, n0:n1]).astype(ml_dtypes.bfloat16),
            "smalls": smalls,
        })
    return in_maps


def _pack_pc(pc_shard):
    """[B,4,12500] -> [128,1568]: row 16g+4b+j = pc[b,j,1568g:1568(g+1)],
    zero-padded to 12544 points (zero points contribute zero error)."""
    pad = np.zeros((B, 4, PAD_N), np.float32)
    pad[:, :, :PTS_PER_CORE] = pc_shard
    v = pad.reshape(B, 4, PC_GROUPS, PC_COLS)
    return np.ascontiguousarray(
        v.transpose(2, 0, 1, 3).reshape(16 * PC_GROUPS, PC_COLS))


def combine_outputs(core_outs):
    """core_outs: [N_CORES, 5] array of per-core partials."""
    core_outs = np.asarray(core_outs, np.float32)
    total = np.float32(core_outs[:, 0].sum())
    lt = np.float32(core_outs[0, 1])
    lr = np.float32(core_outs[0, 2])
    pcb = np.float32(core_outs[:, 3].sum())
    fl = np.float32(core_outs[:, 4].sum())
    return (total, lt, lr, pcb, fl)


def _install_ntff_hook_shim():
    """bass_utils expects antenv.axon_hooks when trace=True under axon;
    this image's antenv lacks it. Provide it and register the ctypes hook."""
    import sys
    import types
    if "antenv.axon_hooks" in sys.modules:
        return
    mod = types.ModuleType("antenv.axon_hooks")
    state = {"hook": None}
    mod.set_axon_ntff_profile_hook = lambda h: state.__setitem__("hook", h)
    mod.get_axon_ntff_profile_hook = lambda: state["hook"]
    sys.modules["antenv.axon_hooks"] = mod
    try:
        import antenv
        antenv.axon_hooks = mod
    except ImportError:
        pass
    try:
        from trn_agent_boot.trn_boot import _ntff_profile_via_ctypes
        mod.set_axon_ntff_profile_hook(
            _ntff_profile_via_ctypes("/opt/axon/libaxon_pjrt.so"))
    except Exception:
        pass


def kernel(point_clouds, target_transl, target_rot, transl_err, rot_err,
           calib_flow_pred, calib_flow_gt, flow_valid):
    global last_results
    from concourse.bass_utils import run_bass_kernel_spmd

    nc = _get_nc()
    in_maps = make_in_maps(point_clouds, target_transl, target_rot,
                           transl_err, rot_err, calib_flow_pred,
                           calib_flow_gt, flow_valid)
    trace = bool(int(os.environ.get("KERNEL_TRACE", "0")))
    kwargs = {}
    if trace:
        _install_ntff_hook_shim()
        kwargs = {"trace": True, "trace_cores": list(range(N_CORES))}
    res = run_bass_kernel_spmd(nc, in_maps, core_ids=list(range(N_CORES)),
                               **kwargs)
    last_results = res
    core_outs = np.stack([res.results[c]["out"][0] for c in range(N_CORES)])
    return combine_outputs(core_outs)


# revision 7
# speedup vs baseline: 2.1508x; 1.0483x over previous
"""Trainium2 Bass kernel for nn_CombinedLoss (pose + point-cloud + flow loss).

Self-contained: accepts FULL inputs, shards across 8 NeuronCores internally,
returns the FULL output (5-tuple of f32 scalars, matching the reference).

Sharding strategy:
  - flow tensors [B,1000,2,32,64]: the loss weights gamma^(999-t) decay so
    fast that iterations with t < 872 contribute < 1e-12 relative to the f32
    result; only the last 128 iterations are loaded.  Those are sharded along
    the iteration axis (16 iters/core) and reshaped to a [128, 2048] tile
    (row = (b, t, c), cols = h*w) so all 128 partitions are used.
  - point_clouds [B,4,N]: sharded along N (12500 pts/core), packed bf16 into
    [128, 1568] so one matmul chain handles all 4 batches.
  - tiny pose tensors: replicated; every core computes the same pose scalars.
Each core emits 5 partial scalars; the host sums partials across cores
(the all-reduce) and takes core 0's value for the replicated pose terms.

Critical path is the pose->E-matrix->pc-matmul chain, so the E-matrix math
is emitted first, spread across gpsimd (pairs) + scalar (scales); the
scalar pose losses (smooth-L1 / quaternion distance) run last, overlapped
with the point-cloud matmuls.
"""

import os

import numpy as np

import concourse.bass as bass
import concourse.bacc as bacc
import concourse.mybir as mybir
import concourse.tile as tile

N_CORES = 8
B = 4
N_PTS = 100000
N_ITERS = 1000
H, W = 32, 64
GAMMA = 0.8

T_KEEP = 128                             # kept flow iterations (tail)
T0_GLOBAL = N_ITERS - T_KEEP             # 872
TPC = T_KEEP // N_CORES                  # 16 iters per core
FROWS = B * TPC * 2                      # 128 rows: (b, t, c)
FREE1 = H * W                            # 2048 cols (h*w)
FLOW_MEAN_DEN = B * 2 * H * W            # 16384 (mean denominator per iter)
PTS_PER_CORE = N_PTS // N_CORES          # 12500
PC_GROUPS = 8                            # point groups -> 128 matmul rows
PC_COLS = 1568                           # padded 12544 / 8 groups
PAD_N = PC_GROUPS * PC_COLS              # 12544 (pads with zero points)

F32 = mybir.dt.float32
BF16 = mybir.dt.bfloat16
AF = mybir.ActivationFunctionType
OP = mybir.AluOpType
AX = mybir.AxisListType

HALF_PI = float(np.pi / 2.0)


def build_nc():
    nc = bacc.Bacc("TRN2", target_bir_lowering=False, debug=False,
                   num_devices=N_CORES)

    pg = nc.dram_tensor("pg", [FROWS, 2 * FREE1], BF16, kind="ExternalInput")
    vv = nc.dram_tensor("vv", [FROWS, FREE1], BF16, kind="ExternalInput")
    wrow = nc.dram_tensor("wrow", [FROWS, 1], F32, kind="ExternalInput")
    pc = nc.dram_tensor("pc", [16 * PC_GROUPS, PC_COLS], BF16,
                        kind="ExternalInput")
    smalls = nc.dram_tensor("smalls", [B, 14], F32, kind="ExternalInput")
    out = nc.dram_tensor("out", [1, 5], F32, kind="ExternalOutput")

    with tile.TileContext(nc) as tc:
        _body(nc, tc, pg, vv, wrow, pc, smalls, out)
    nc.compile()
    return nc


def _body(nc, tc, pg, vv, wrow, pc, smalls, out):
    with (
        tc.tile_pool(name="small", bufs=1) as small,
        tc.tile_pool(name="flow", bufs=1) as flow,
        tc.tile_pool(name="pcpool", bufs=1) as pcpool,
        tc.tile_pool(name="pwork", bufs=3) as pwork,
        tc.tile_pool(name="psum_d", bufs=2, space="PSUM") as psum_d,
        tc.tile_pool(name="psum_e", bufs=2, space="PSUM") as psum_e,
        tc.tile_pool(name="psum_s", bufs=1, space="PSUM") as psum_s,
        tc.tile_pool(name="dram", bufs=1, space="DRAM") as dram,
    ):
        cnt = [0]

        def st(p_, f_, tag=None, dt=F32):
            cnt[0] += 1
            nm = tag or f"s{cnt[0]}"
            return small.tile([p_, f_], dt, name=nm, tag=nm)

        # ---------------- DMA triggers (spread across queues) --------------
        sm = st(B, 14, tag="sm")
        nc.sync.dma_start(sm[:], smalls[:])          # pose inputs first
        pcp = pcpool.tile([128, PC_COLS], BF16, tag="pcp")
        nc.sync.dma_start(pcp[:], pc[:])
        pgt = flow.tile([FROWS, 2 * FREE1], BF16, tag="pgt")
        nc.sync.dma_start(pgt[:64, :], pg[:64, :])
        nc.scalar.dma_start(pgt[64:, :], pg[64:, :])
        wt = st(FROWS, 1, tag="wt")
        nc.scalar.dma_start(wt[:], wrow[:])
        vvt = flow.tile([FROWS, FREE1], BF16, tag="vvt")
        nc.gpsimd.dma_start(vvt[:], vv[:])

        # lhsT3 [128,32] static: ones at (16g+4b+i, 4g+b) -- coordinate sum
        import ml_dtypes
        l3_np = np.zeros((128, 32), dtype=ml_dtypes.bfloat16)
        for g in range(PC_GROUPS):
            for b in range(B):
                for i in range(4):
                    l3_np[16 * g + 4 * b + i, 4 * g + b] = 1.0
        l3_dram = nc.inline_tensor(np.asarray(l3_np), name="l3_const")
        lhsT3 = st(128, 32, tag="lhsT3", dt=BF16)
        nc.gpsimd.dma_start(lhsT3[:], l3_dram[:])

        tt_s, tr_s, te_s, re_s = sm[:, 0:3], sm[:, 3:7], sm[:, 7:10], sm[:, 10:14]

        # ============ E-matrix path (critical: feeds pc matmuls) ===========
        # normalized quaternions
        def qnormalize(q_s):
            sq = st(B, 4)
            nc.gpsimd.tensor_mul(sq[:], q_s[:], q_s[:])
            n2 = st(B, 1)
            nc.vector.tensor_reduce(n2[:], sq[:], axis=AX.X, op=OP.add)
            nr = st(B, 1)
            nc.scalar.activation(nr[:], n2[:], AF.Sqrt)
            inv = st(B, 1)
            nc.vector.reciprocal(inv[:], nr[:])
            qn = st(B, 4)
            nc.gpsimd.tensor_scalar(qn[:], q_s[:], inv[:], None, OP.mult)
            return qn

        e = qnormalize(re_s)   # normalized rot_err
        f = qnormalize(tr_s)   # normalized target_rot

        # qm = conj(e) x f  (so R(qm) = R(e)^T R(f))
        F0 = st(B, 4)
        F1 = st(B, 4)
        F2 = st(B, 4)
        F3 = st(B, 4)
        nc.gpsimd.tensor_scalar(F0[:], f[:], e[:, 0:1], None, OP.mult)
        nc.gpsimd.tensor_scalar(F1[:], f[:], e[:, 1:2], None, OP.mult)
        nc.gpsimd.tensor_scalar(F2[:], f[:], e[:, 2:3], None, OP.mult)
        nc.gpsimd.tensor_scalar(F3[:], f[:], e[:, 3:4], None, OP.mult)
        Q = st(B, 4)  # qm = (gw, gx, gy, gz)
        nc.gpsimd.tensor_add(Q[:, 0:1], F0[:, 0:1], F1[:, 1:2])
        nc.gpsimd.tensor_add(Q[:, 0:1], Q[:, 0:1], F2[:, 2:3])
        nc.gpsimd.tensor_add(Q[:, 0:1], Q[:, 0:1], F3[:, 3:4])
        nc.gpsimd.tensor_sub(Q[:, 1:2], F0[:, 1:2], F1[:, 0:1])
        nc.gpsimd.tensor_sub(Q[:, 1:2], Q[:, 1:2], F2[:, 3:4])
        nc.gpsimd.tensor_add(Q[:, 1:2], Q[:, 1:2], F3[:, 2:3])
        nc.gpsimd.tensor_add(Q[:, 2:3], F0[:, 2:3], F1[:, 3:4])
        nc.gpsimd.tensor_sub(Q[:, 2:3], Q[:, 2:3], F2[:, 0:1])
        nc.gpsimd.tensor_sub(Q[:, 2:3], Q[:, 2:3], F3[:, 1:2])
        nc.gpsimd.tensor_sub(Q[:, 3:4], F0[:, 3:4], F1[:, 2:3])
        nc.gpsimd.tensor_add(Q[:, 3:4], Q[:, 3:4], F2[:, 1:2])
        nc.gpsimd.tensor_sub(Q[:, 3:4], Q[:, 3:4], F3[:, 0:1])

        # A = M3 - I entries, E layout [B, 4j+i]
        G1 = st(B, 4)
        G2 = st(B, 4)
        G3 = st(B, 4)
        nc.gpsimd.tensor_scalar(G1[:], Q[:], Q[:, 1:2], None, OP.mult)
        nc.gpsimd.tensor_scalar(G2[:], Q[:], Q[:, 2:3], None, OP.mult)
        nc.gpsimd.tensor_scalar(G3[:], Q[:], Q[:, 3:4], None, OP.mult)
        E = st(B, 16)
        nc.gpsimd.memset(E[:], 0.0)

        def emit(col, p_a, p_b, sub, scale2, plus1=False):
            # pair add/sub on gpsimd; scale+bias on the (idle) scalar engine
            s = st(B, 1)
            if sub:
                nc.gpsimd.tensor_sub(s[:], p_a, p_b)
            else:
                nc.gpsimd.tensor_add(s[:], p_a, p_b)
            nc.scalar.activation(E[:, col:col + 1], s[:], AF.Copy,
                                 bias=1.0 if plus1 else 0.0, scale=scale2)

        emit(0, G2[:, 2:3], G3[:, 3:4], False, -2.0)            # A00=-2(y2+z2)
        emit(5, G1[:, 1:2], G3[:, 3:4], False, -2.0)            # A11=-2(x2+z2)
        emit(10, G1[:, 1:2], G2[:, 2:3], False, -2.0)           # A22=-2(x2+y2)
        emit(4, G1[:, 2:3], G3[:, 0:1], True, 2.0)              # A01=2(xy-wz)
        emit(8, G1[:, 3:4], G2[:, 0:1], False, 2.0)             # A02=2(xz+wy)
        emit(1, G1[:, 2:3], G3[:, 0:1], False, 2.0)             # A10=2(xy+wz)
        emit(9, G2[:, 3:4], G1[:, 0:1], True, 2.0)              # A12=2(yz-wx)
        emit(2, G1[:, 3:4], G2[:, 0:1], True, 2.0)              # A20=2(xz-wy)
        emit(6, G2[:, 3:4], G1[:, 0:1], False, 2.0)             # A21=2(yz+wx)

        # translation column: Mt = R(e)^T (tt - te) into E[:, 12:15]
        Hx = st(B, 4)
        Hy = st(B, 4)
        Hz = st(B, 4)
        nc.gpsimd.tensor_scalar(Hx[:], e[:], e[:, 1:2], None, OP.mult)
        nc.gpsimd.tensor_scalar(Hy[:], e[:], e[:, 2:3], None, OP.mult)
        nc.gpsimd.tensor_scalar(Hz[:], e[:], e[:, 3:4], None, OP.mult)
        row0 = st(B, 3)
        row1 = st(B, 3)
        row2 = st(B, 3)

        def rentry(dst, p_a, p_b, sub, scale2, plus1):
            s = st(B, 1)
            if sub:
                nc.gpsimd.tensor_sub(s[:], p_a, p_b)
            else:
                nc.gpsimd.tensor_add(s[:], p_a, p_b)
            nc.scalar.activation(dst, s[:], AF.Copy,
                                 bias=1.0 if plus1 else 0.0, scale=scale2)

        rentry(row0[:, 0:1], Hy[:, 2:3], Hz[:, 3:4], False, -2.0, True)
        rentry(row0[:, 1:2], Hx[:, 2:3], Hz[:, 0:1], True, 2.0, False)
        rentry(row0[:, 2:3], Hx[:, 3:4], Hy[:, 0:1], False, 2.0, False)
        rentry(row1[:, 0:1], Hx[:, 2:3], Hz[:, 0:1], False, 2.0, False)
        rentry(row1[:, 1:2], Hx[:, 1:2], Hz[:, 3:4], False, -2.0, True)
        rentry(row1[:, 2:3], Hy[:, 3:4], Hx[:, 0:1], True, 2.0, False)
        rentry(row2[:, 0:1], Hx[:, 3:4], Hy[:, 0:1], True, 2.0, False)
        rentry(row2[:, 1:2], Hy[:, 3:4], Hx[:, 0:1], False, 2.0, False)
        rentry(row2[:, 2:3], Hx[:, 1:2], Hy[:, 2:3], False, -2.0, True)

        u = st(B, 3)
        nc.gpsimd.tensor_sub(u[:], tt_s, te_s)
        nc.gpsimd.tensor_scalar(E[:, 12:15], row0[:], u[:, 0:1], None, OP.mult)
        nc.vector.scalar_tensor_tensor(E[:, 12:15], row1[:], u[:, 1:2],
                                       E[:, 12:15], OP.mult, OP.add)
        nc.vector.scalar_tensor_tensor(E[:, 12:15], row2[:], u[:, 2:3],
                                       E[:, 12:15], OP.mult, OP.add)

        # ---------------- flow (vector engine; DMA-gated) ----------------
        d_t = flow.tile([FROWS, FREE1], BF16, tag="d")
        nc.vector.tensor_sub(d_t[:], pgt[:, 0:FREE1], pgt[:, FREE1:2 * FREE1])
        dv_t = flow.tile([FROWS, FREE1], BF16, tag="dv")
        nc.vector.tensor_mul(dv_t[:], d_t[:], vvt[:])
        dum = flow.tile([FROWS, FREE1], BF16, tag="dum")
        rs2 = st(FROWS, 1, tag="rs2")
        # |w*dv| = w*|dv| (w>0), summed over the free axis
        nc.scalar.activation(dum[:], dv_t[:], AF.Abs, scale=wt[:],
                             accum_out=rs2[:])

        # --------- build lhsT2 [128,128] bf16: A_b[i,j] at (16g+4b+j, 16g+4b+i)
        # one DMA to DRAM, one 4D-AP dram->dram scatter over an inline-zero
        # base, one load back.  addr = 2064*g + 516*b + 128*j + i.
        E_bf = st(B, 16, tag="E_bf", dt=BF16)
        nc.gpsimd.tensor_scalar(E_bf[:], E[:], 1.0, None, OP.mult)
        e_flat = dram.tile([B, 16], BF16)
        nc.gpsimd.dma_start(e_flat[:], E_bf[:])
        l2d_z = nc.inline_tensor(
            np.zeros((128, 128), dtype=ml_dtypes.bfloat16), name="l2d_zero")
        sap = bass.AP(e_flat[:].tensor, 0, [[0, 8], [16, 4], [4, 4], [1, 4]])
        dap = bass.AP(l2d_z[:].tensor, 0, [[2064, 8], [516, 4], [128, 4], [1, 4]])
        nc.gpsimd.dma_start(dap, sap)
        lhsT2 = st(128, 128, tag="lhsT2", dt=BF16)
        nc.gpsimd.dma_start(lhsT2[:], l2d_z[:])

        # ---------------- point-cloud: K=128 matmuls over [128,1568] ------
        acc32 = st(32, 1, tag="acc32")
        nc.gpsimd.memset(acc32[:], 0.0)
        dsq = pcpool.tile([128, PC_COLS], BF16, tag="dsq")
        col_chunks = [(0, 512), (512, 1024), (1024, 1536), (1536, PC_COLS)]
        for c0, c1 in col_chunks:
            dps = psum_d.tile([128, 512], F32, tag="dps")
            nc.tensor.matmul(dps[:, :c1 - c0], lhsT2[:], pcp[:, c0:c1],
                             start=True, stop=True)
            nc.scalar.activation(dsq[:, c0:c1], dps[:, :c1 - c0], AF.Square)
        for c0, c1 in col_chunks:
            e2 = psum_e.tile([32, 512], F32, tag="e2")
            nc.tensor.matmul(e2[:, :c1 - c0], lhsT3[:], dsq[:, c0:c1],
                             start=True, stop=True)
            errt = pwork.tile([32, 512], F32, tag="errt")
            ers = pwork.tile([32, 1], F32, tag="ers")
            nc.scalar.activation(errt[:, :c1 - c0], e2[:, :c1 - c0], AF.Sqrt,
                                 accum_out=ers[:])
            nc.vector.tensor_add(acc32[:], acc32[:], ers[:])

        # ============ scalar pose losses (off critical path) ==============
        # smooth-L1 translation loss
        d = st(B, 3)
        nc.gpsimd.tensor_sub(d[:], te_s, tt_s)
        a = st(B, 3)
        nc.scalar.activation(a[:], d[:], AF.Abs)
        d2 = st(B, 3)
        nc.gpsimd.tensor_mul(d2[:], d[:], d[:])
        half_d2 = st(B, 3)
        nc.gpsimd.tensor_scalar(half_d2[:], d2[:], 0.5, None, OP.mult)
        am = st(B, 3)
        nc.gpsimd.tensor_scalar(am[:], a[:], 0.5, None, OP.subtract)
        mlt = st(B, 3, dt=mybir.dt.int32)
        nc.vector.tensor_scalar(mlt[:], a[:], 1.0, None, OP.is_lt)
        sl1 = st(B, 3)
        nc.vector.select(sl1[:], mlt[:], half_d2[:], am[:])
        lt_row = st(B, 1)  # per-batch smooth-l1 row sums
        nc.vector.tensor_reduce(lt_row[:], sl1[:], axis=AX.X, op=OP.add)

        # loss_rot (quaternion distance, RAW quats)
        P0 = st(B, 4)
        P1 = st(B, 4)
        P2 = st(B, 4)
        P3 = st(B, 4)
        nc.gpsimd.tensor_scalar(P0[:], tr_s, sm[:, 10:11], None, OP.mult)
        nc.gpsimd.tensor_scalar(P1[:], tr_s, sm[:, 11:12], None, OP.mult)
        nc.gpsimd.tensor_scalar(P2[:], tr_s, sm[:, 12:13], None, OP.mult)
        nc.gpsimd.tensor_scalar(P3[:], tr_s, sm[:, 13:14], None, OP.mult)
        tw = st(B, 1)
        tx = st(B, 1)
        ty = st(B, 1)
        tz = st(B, 1)
        nc.gpsimd.tensor_add(tw[:], P0[:, 0:1], P1[:, 1:2])
        nc.gpsimd.tensor_add(tw[:], tw[:], P2[:, 2:3])
        nc.gpsimd.tensor_add(tw[:], tw[:], P3[:, 3:4])
        nc.gpsimd.tensor_sub(tx[:], P1[:, 0:1], P0[:, 1:2])
        nc.gpsimd.tensor_add(tx[:], tx[:], P3[:, 2:3])
        nc.gpsimd.tensor_sub(tx[:], tx[:], P2[:, 3:4])
        nc.gpsimd.tensor_sub(ty[:], P2[:, 0:1], P0[:, 2:3])
        nc.gpsimd.tensor_add(ty[:], ty[:], P1[:, 3:4])
        nc.gpsimd.tensor_sub(ty[:], ty[:], P3[:, 1:2])
        nc.gpsimd.tensor_sub(tz[:], P2[:, 1:2], P0[:, 3:4])
        nc.gpsimd.tensor_add(tz[:], tz[:], P3[:, 0:1])
        nc.gpsimd.tensor_sub(tz[:], tz[:], P1[:, 2:3])
        vn2 = st(B, 1)
        nc.gpsimd.tensor_mul(vn2[:], tx[:], tx[:])
        nc.vector.scalar_tensor_tensor(vn2[:], ty[:], ty[:], vn2[:], OP.mult, OP.add)
        nc.vector.scalar_tensor_tensor(vn2[:], tz[:], tz[:], vn2[:], OP.mult, OP.add)
        vn = st(B, 1)
        nc.scalar.activation(vn[:], vn2[:], AF.Sqrt)
        aw = st(B, 1)
        nc.scalar.activation(aw[:], tw[:], AF.Abs)
        mx = st(B, 1)
        nc.vector.tensor_max(mx[:], vn[:], aw[:])
        mn = st(B, 1)
        nc.vector.tensor_tensor(mn[:], vn[:], aw[:], OP.min)
        rec = st(B, 1)
        nc.vector.reciprocal(rec[:], mx[:])
        ratio = st(B, 1)
        nc.gpsimd.tensor_mul(ratio[:], mn[:], rec[:])
        ang = st(B, 1)
        nc.scalar.activation(ang[:], ratio[:], AF.Arctan)
        mflip = st(B, 1, dt=mybir.dt.int32)
        nc.vector.tensor_tensor(mflip[:], vn[:], aw[:], OP.is_gt)
        alt = st(B, 1)
        nc.gpsimd.tensor_scalar(alt[:], ang[:], -1.0, HALF_PI, OP.mult, OP.add)
        rot = st(B, 1)  # atan2 per batch
        nc.vector.select(rot[:], mflip[:], alt[:], ang[:])

        # ---------------- final reductions ----------------
        ones128 = st(128, 1, tag="ones128")
        nc.gpsimd.memset(ones128[:], 1.0)
        ones4 = st(B, 1, tag="ones4")
        nc.gpsimd.memset(ones4[:], 1.0)
        ones32 = st(32, 1, tag="ones32")
        nc.gpsimd.memset(ones32[:], 1.0)
        ps = psum_s.tile([1, 4], F32, tag="ps")
        nc.tensor.matmul(ps[:, 0:1], rs2[:], ones128[:], start=True, stop=True)
        nc.tensor.matmul(ps[:, 1:2], acc32[:], ones32[:], start=True, stop=True)
        nc.tensor.matmul(ps[:, 2:3], lt_row[:], ones4[:], start=True, stop=True)
        nc.tensor.matmul(ps[:, 3:4], rot[:], ones4[:], start=True, stop=True)

        out5 = st(1, 5, tag="out5")
        # loss_transl = sum/4 ; loss_rot = 2*sum/4 ; pc = sum/(B*N) ; flow = sum
        nc.scalar.mul(out5[:, 1:2], ps[:, 2:3], 0.25)
        nc.scalar.mul(out5[:, 2:3], ps[:, 3:4], 0.5)
        nc.scalar.mul(out5[:, 3:4], ps[:, 1:2], 1.0 / (B * N_PTS))
        nc.scalar.copy(out5[:, 4:5], ps[:, 0:1])
        t1 = st(1, 1)
        t2 = st(1, 1)
        nc.gpsimd.tensor_add(t1[:], out5[:, 1:2], out5[:, 2:3])
        nc.gpsimd.tensor_add(t2[:], out5[:, 3:4], out5[:, 4:5])
        nc.gpsimd.tensor_scalar(t1[:], t1[:], 0.5 / N_CORES, None, OP.mult)
        nc.vector.scalar_tensor_tensor(out5[:, 0:1], t2[:], 0.5, t1[:],
                                       OP.mult, OP.add)
        nc.sync.dma_start(out[:], out5[:])


_CACHE = {}
last_results = None


def _get_nc():
    if "nc" not in _CACHE:
        _CACHE["nc"] = build_nc()
    return _CACHE["nc"]


def make_in_maps(point_clouds, target_transl, target_rot, transl_err, rot_err,
                 calib_flow_pred, calib_flow_gt, flow_valid):
    import ml_dtypes
    point_clouds = np.asarray(point_clouds, np.float32)
    calib_flow_pred = np.asarray(calib_flow_pred, np.float32)
    calib_flow_gt = np.asarray(calib_flow_gt, np.float32)
    flow_valid = np.asarray(flow_valid, np.float32)
    tt = np.ascontiguousarray(np.asarray(target_transl, np.float32))
    tr = np.ascontiguousarray(np.asarray(target_rot, np.float32))
    te = np.ascontiguousarray(np.asarray(transl_err, np.float32))
    re = np.ascontiguousarray(np.asarray(rot_err, np.float32))

    w_full = (GAMMA ** (N_ITERS - 1 - np.arange(N_ITERS, dtype=np.float64)))
    w_full = (w_full / FLOW_MEAN_DEN).astype(np.float32)

    smalls = np.concatenate([tt, tr, te, re], axis=1).astype(np.float32)
    in_maps = []
    for c in range(N_CORES):
        t0 = T0_GLOBAL + c * TPC
        t1 = t0 + TPC
        n0, n1 = c * PTS_PER_CORE, (c + 1) * PTS_PER_CORE
        p_s = calib_flow_pred[:, t0:t1].reshape(FROWS, FREE1)
        g_s = calib_flow_gt[:, t0:t1].reshape(FROWS, FREE1)
        vv_s = np.repeat(flow_valid[:, t0:t1], 2, axis=2).reshape(FROWS, FREE1)
        # row p = b*32 + ti*2 + k  ->  w = w_full[t0+ti]
        w_s = np.tile(np.repeat(w_full[t0:t1], 2), B).reshape(FROWS, 1)
        in_maps.append({
            "pg": np.ascontiguousarray(
                np.concatenate([p_s, g_s], axis=1)).astype(ml_dtypes.bfloat16),
            "vv": np.ascontiguousarray(vv_s).astype(ml_dtypes.bfloat16),
            "wrow": np.ascontiguousarray(w_s.astype(np.float32)),
            "pc": _pack_pc(point_clouds[:, :, n0:n1]).astype(ml_dtypes.bfloat16),
            "smalls": smalls,
        })
    return in_maps


def _pack_pc(pc_shard):
    """[B,4,12500] -> [128,1568]: row 16g+4b+j = pc[b,j,1568g:1568(g+1)],
    zero-padded to 12544 points (zero points contribute zero error)."""
    pad = np.zeros((B, 4, PAD_N), np.float32)
    pad[:, :, :PTS_PER_CORE] = pc_shard
    v = pad.reshape(B, 4, PC_GROUPS, PC_COLS)
    return np.ascontiguousarray(
        v.transpose(2, 0, 1, 3).reshape(16 * PC_GROUPS, PC_COLS))


def combine_outputs(core_outs):
    """core_outs: [N_CORES, 5] array of per-core partials."""
    core_outs = np.asarray(core_outs, np.float32)
    total = np.float32(core_outs[:, 0].sum())
    lt = np.float32(core_outs[0, 1])
    lr = np.float32(core_outs[0, 2])
    pcb = np.float32(core_outs[:, 3].sum())
    fl = np.float32(core_outs[:, 4].sum())
    return (total, lt, lr, pcb, fl)


def _install_ntff_hook_shim():
    """bass_utils expects antenv.axon_hooks when trace=True under axon;
    this image's antenv lacks it. Provide it and register the ctypes hook."""
    import sys
    import types
    if "antenv.axon_hooks" in sys.modules:
        return
    mod = types.ModuleType("antenv.axon_hooks")
    state = {"hook": None}
    mod.set_axon_ntff_profile_hook = lambda h: state.__setitem__("hook", h)
    mod.get_axon_ntff_profile_hook = lambda: state["hook"]
    sys.modules["antenv.axon_hooks"] = mod
    try:
        import antenv
        antenv.axon_hooks = mod
    except ImportError:
        pass
    try:
        from trn_agent_boot.trn_boot import _ntff_profile_via_ctypes
        mod.set_axon_ntff_profile_hook(
            _ntff_profile_via_ctypes("/opt/axon/libaxon_pjrt.so"))
    except Exception:
        pass


def kernel(point_clouds, target_transl, target_rot, transl_err, rot_err,
           calib_flow_pred, calib_flow_gt, flow_valid):
    global last_results
    from concourse.bass_utils import run_bass_kernel_spmd

    nc = _get_nc()
    in_maps = make_in_maps(point_clouds, target_transl, target_rot,
                           transl_err, rot_err, calib_flow_pred,
                           calib_flow_gt, flow_valid)
    trace = bool(int(os.environ.get("KERNEL_TRACE", "0")))
    kwargs = {}
    if trace:
        _install_ntff_hook_shim()
        kwargs = {"trace": True, "trace_cores": list(range(N_CORES))}
    res = run_bass_kernel_spmd(nc, in_maps, core_ids=list(range(N_CORES)),
                               **kwargs)
    last_results = res
    core_outs = np.stack([res.results[c]["out"][0] for c in range(N_CORES)])
    return combine_outputs(core_outs)
